# revision 1
# baseline (speedup 1.0000x reference)
"""Trainium2 Bass kernel for CorrespondenceFeatGeneration (patch-correlation argmax flow).

Math (per image, mirrors the reference):
  fin, fref: (256, 64, 64) -> unit-normalize each pixel across channels.
  corr[p, q] = <3x3 patch of fref at p, 3x3 patch of fin at q>   (2304-dim dot)
  max_idx[q] = argmax_p corr[p, q]  (first-max tie-break)
  flow[q] = (px - qx, py - qy), zero-padded to 64x64, then 9 shifted copies.

The reference additionally row-normalizes ref patches by (||row|| + 1e-5); every
row norm is exactly sqrt(9) up to 1e-7 because pixels are unit vectors, so the
scale is uniform-positive and argmax-invariant -> skipped.

Device strategy (8 NeuronCores, SPMD):
  - Shard q (input patch positions incl. 64-grid padding): 2 images x 4 blocks
    of 1024 q-pixels. Each core computes argmax over ALL ref positions for its
    q's -> no cross-core reduction.
  - corr tile (128 q, 496 p-chunk) = sum over 9 patch offsets s and 2 channel
    halves of UinT[c, q+s] @ Uref[c, p+s]: shifted *views* of channel-major
    feature matrices -- no 9x patch materialization.
  - Precision: fp16 hi/lo split, 3 terms (hi*hi + lo*hi + hi*lo) accumulated in
    fp32 PSUM. Max error ~1e-6, well under the min top-2 corr gap (~3e-5).
    (fp32 matmul is 4 cyc/row; fp32r is tf32-like and flips argmaxes; fp16
    subnormals are exact on the PE -- probed.)
  - Argmax: VectorE max / max_index (first occurrence == reference tie-break),
    invalid p columns (px>=62) masked to -1e30 first.
Host: unit-normalize, fp16 split, pad; decode idx -> flow + 9 shifts.
"""

import numpy as np

H = W = 64
C = 256
HP = H * W          # 4096 pixel positions per image
PW_PAD = 4224       # padded ref pixel columns (>= 4096 + 130)
QW_PAD = 4352       # padded input pixel columns for windowing
QBLK = 1024         # q positions per core
QWIN = 1280         # per-core input window width (1024 + 130, padded to 1280)
NQT = 8             # q-tiles of 128 per core
NPC = 8             # p-chunks
PC = 496            # p-chunk width (8 * 496 = 3968 >= 62*64 - 1 valid max p)
PW = NPC * PC       # 3968
CPW = NPC * 512     # 4096: C3 (x-patch correlation) width, 8 full PSUM banks
SHIFTS = [64 * dy + dx for dy in range(3) for dx in range(3)]

_RUNNER = None


def _build_runner():
    import sys
    if '/opt/trn_rl_repo' not in sys.path:
        sys.path.insert(0, '/opt/trn_rl_repo')
    import concourse.bass as bass  # noqa: F401
    import concourse.tile as tile
    from concourse import bacc, mybir
    from concourse.bass_utils import run_bass_kernel_spmd

    f16 = mybir.dt.float16
    f32 = mybir.dt.float32
    u32 = mybir.dt.uint32

    nc = bacc.Bacc("TRN2", target_bir_lowering=False, debug=False, num_devices=8)

    d_in = {}
    for nm in ("uin_hi", "uin_lo"):
        d_in[nm] = nc.dram_tensor(nm, [C, QWIN], f16, kind="ExternalInput").ap()
    for nm in ("uref_hi", "uref_lo"):
        d_in[nm] = nc.dram_tensor(nm, [C, PW_PAD], f16, kind="ExternalInput").ap()
    idx_d = nc.dram_tensor("idx", [128, NQT], u32, kind="ExternalOutput").ap()
    vals_d = nc.dram_tensor("vals", [128, 2 * NQT], f32, kind="ExternalOutput").ap()
    junk_d = nc.dram_tensor("junk", [1, 8], f32, kind="ExternalOutput").ap()

    TERMS = [("uin_hi", "uref_hi"), ("uin_lo", "uref_hi"), ("uin_hi", "uref_lo")]

    with tile.TileContext(nc) as tc:
        with tc.tile_pool(name="const", bufs=1) as cpool, \
             tc.tile_pool(name="corr", bufs=2) as corrpool, \
             tc.tile_pool(name="small", bufs=2) as spool, \
             tc.tile_pool(name="stage", bufs=1) as stpool, \
             tc.tile_pool(name="ps", bufs=1, space="PSUM") as ps:

            # Issue order matters: the first real matmuls read uin_hi/uref_hi,
            # so land those before the lo-split halves to start the PE early.
            ins = {}
            for nm, w in (("uin_hi", QWIN), ("uref_hi", PW_PAD),
                          ("uin_lo", QWIN), ("uref_lo", PW_PAD)):
                for ch in range(2):
                    t = cpool.tile([128, w], f16, tag=f"{nm}{ch}")
                    nc.sync.dma_start(t[:], d_in[nm][128 * ch:128 * (ch + 1), :])
                    ins[(nm, ch)] = t

            # Wait-absorber matmuls: walrus allows only one sync wait on the
            # LDW side of a matmul. Absorb the (small, issued-first) uin-side
            # DMA waits here; the term/ch step order below then introduces at
            # most one new uref region per step, so real matmuls never carry
            # two fresh DMA waits -- and the PE starts as soon as the first
            # uref half arrives instead of after all input DMAs.
            junk_ps = ps.tile([128, PC], f32, tag="bank0")
            regions = [ins[(nm, ch)]
                       for nm in ("uin_hi", "uin_lo")
                       for ch in range(2)]
            for i, r in enumerate(regions):
                nc.tensor.matmul(junk_ps[:1, :8], r[:, :1], r[:, :8],
                                 start=(i == 0), stop=(i == len(regions) - 1))
            junk_sb = stpool.tile([128, 8], f32, tag="junk")
            nc.vector.tensor_copy(junk_sb[:1, :8], junk_ps[:1, :8])

            idx_stage = stpool.tile([128, NQT], u32, tag="idxs")
            vals_stage = stpool.tile([128, 2 * NQT], f32, tag="valss")

            # C3[q, p] = x-patch (1x3) correlation = sum over dx, c of
            #   Uin[c, q+dx] * Uref[c, p+dx]   (fp16 3-term split, fp32 PSUM)
            # corr[q, p] = sum over dy of C3[q + 64*dy, p + 64*dy]
            # -> PE work drops 3x vs folding all 9 offsets into the matmul;
            #    the dy-sum is 2 fp32 DVE adds (dy=2 is a whole-tile-aligned
            #    view; dy=1 needs one small partition-rebasing DMA stage).
            c3_tiles = {}
            for ct in range(NQT + 1):
                banks = [ps.tile([128, 512], f32, name=f"bank{pc}", tag=f"bank{pc}")
                         for pc in range(NPC)]
                n_steps = len(TERMS) * 3 * 2
                step = 0
                for (anm, bnm) in TERMS:
                    for ch in range(2):
                        for dx in range(3):
                            lhsT = ins[(anm, ch)][:, ct * 128 + dx: ct * 128 + dx + 128]
                            for pc in range(NPC):
                                rhs = ins[(bnm, ch)][:, pc * 512 + dx: pc * 512 + dx + 512]
                                nc.tensor.matmul(banks[pc][:], lhsT, rhs,
                                                 start=(step == 0),
                                                 stop=(step == n_steps - 1))
                            step += 1
                c3 = corrpool.tile([128, CPW], f32, name="c3", tag="c3", bufs=3)
                for pc in range(NPC):
                    nc.scalar.activation(c3[:, pc * 512:(pc + 1) * 512], banks[pc][:],
                                         mybir.ActivationFunctionType.Copy)
                c3_tiles[ct] = c3
                if ct == 0:
                    continue

                t = ct - 1
                prev, cur = c3_tiles[t], c3_tiles[ct]
                # dy=1 operand: C3 rows [t*128+64, +128) with +64 column offset
                stage = corrpool.tile([128, PW], f32, name="stage", tag="stage", bufs=2)
                nc.sync.dma_start(stage[0:64, :], prev[64:128, 64:64 + PW])
                nc.sync.dma_start(stage[64:128, :], cur[0:64, 64:64 + PW])
                corr = corrpool.tile([128, PW], f32, name="corr", tag="corr", bufs=2)
                nc.vector.tensor_add(corr[:], prev[:, 0:PW], stage[:])
                nc.vector.tensor_add(corr[:], corr[:], cur[:, 128:128 + PW])
                # mask invalid ref columns (px in {62, 63}); PW = 62*64 exactly
                mask_ap = corr[:].rearrange("p (a b) -> p a b", b=64)[:, :, 62:64]
                nc.vector.memset(mask_ap, -1e30)

                mx = spool.tile([128, 8], f32, tag="mx")
                mi = spool.tile([128, 8], u32, tag="mi")
                nc.vector.max(mx[:], corr[:])
                nc.vector.max_index(mi[:], mx[:], corr[:])
                nc.vector.tensor_copy(idx_stage[:, t:t + 1], mi[:, 0:1])
                nc.vector.tensor_copy(vals_stage[:, 2 * t:2 * t + 2], mx[:, 0:2])
                del c3_tiles[t]

            nc.sync.dma_start(idx_d[:], idx_stage[:])
            nc.sync.dma_start(vals_d[:], vals_stage[:])
            nc.sync.dma_start(junk_d[:], junk_sb[:1, :8])

    nc.compile()
    return nc, run_bass_kernel_spmd


def _unit_pixels(f):
    # f: (C, H, W) float32; unit L2 norm per pixel across channels (fp32 math)
    n = np.sqrt(np.sum(f * f, axis=0, keepdims=True, dtype=np.float32))
    return (f / np.maximum(n, np.float32(1e-12))).astype(np.float32)


def _split_f16(a):
    hi = a.astype(np.float16)
    lo = (a - hi.astype(np.float32)).astype(np.float16)
    return hi, lo


def kernel(dense_features1, dense_features2, img_ref_hr):
    global _RUNNER
    if _RUNNER is None:
        _RUNNER = _build_runner()
    nc, run_spmd = _RUNNER

    f1 = np.asarray(dense_features1, dtype=np.float32)  # input features (b,C,H,W)
    f2 = np.asarray(dense_features2, dtype=np.float32)  # ref features
    B = f1.shape[0]
    assert B == 2 and f1.shape[1:] == (C, H, W)

    in_maps = []
    per_img = []
    for b in range(B):
        fin_u = _unit_pixels(f1[b]).reshape(C, HP)
        fref_u = _unit_pixels(f2[b]).reshape(C, HP)
        uin = np.zeros((C, QW_PAD), np.float32)
        uin[:, :HP] = fin_u
        uref = np.zeros((C, PW_PAD), np.float32)
        uref[:, :HP] = fref_u
        uin_hi, uin_lo = _split_f16(uin)
        uref_hi, uref_lo = _split_f16(uref)
        per_img.append((uin_hi, uin_lo, uref_hi, uref_lo))

    for core in range(8):
        b, qblk = divmod(core, 4)
        uin_hi, uin_lo, uref_hi, uref_lo = per_img[b]
        q0 = qblk * QBLK
        in_maps.append({
            "uin_hi": np.ascontiguousarray(uin_hi[:, q0:q0 + QWIN]),
            "uin_lo": np.ascontiguousarray(uin_lo[:, q0:q0 + QWIN]),
            "uref_hi": uref_hi,
            "uref_lo": uref_lo,
        })

    results = run_spmd(nc, in_maps, list(range(8))).results

    # Decode: idx_stage[part, tile] = argmax p-flat (64-grid) for
    # q_local = tile*128 + part, global q = core_q0 + q_local.
    out = np.zeros((B, 9, H, W, 2), np.float32)
    qx_grid = np.arange(62, dtype=np.float32)[None, :]
    qy_grid = np.arange(62, dtype=np.float32)[:, None]
    for b in range(B):
        idx_full = np.zeros(HP, np.int64)
        for qblk in range(4):
            r = results[b * 4 + qblk]["idx"]  # (128, NQT) uint32
            idx_full[qblk * QBLK:(qblk + 1) * QBLK] = r.T.reshape(-1)
        idx_grid = idx_full.reshape(H, W)[:62, :62]
        py = (idx_grid // 64).astype(np.float32)
        px = (idx_grid % 64).astype(np.float32)
        flow = np.zeros((H, W, 2), np.float32)
        flow[:62, :62, 0] = px - qx_grid
        flow[:62, :62, 1] = py - qy_grid
        for k, (i, j) in enumerate([(i, j) for i in range(3) for j in range(3)]):
            out[b, k, i:, j:, :] = flow[:H - i, :W - j, :]
    return out



# revision 2
# speedup vs baseline: 1.1690x; 1.1690x over previous
"""Trainium2 Bass kernel for CorrespondenceFeatGeneration (patch-correlation argmax flow).

Math (per image, mirrors the reference):
  fin, fref: (256, 64, 64) -> unit-normalize each pixel across channels.
  corr[q, p] = <3x3 patch of fref at p, 3x3 patch of fin at q>   (2304-dim dot)
             = sum_{s in {0,1,2,64,65,66,128,129,130}} G[q+s, p+s],
               G[a, b] = <u_a, v_b>  (pixel correlation, K=256)
  max_idx[q] = argmax_{valid p} corr[q, p]  (first-max tie-break)

v2 structure (vs the dx-folded-into-PE baseline at ~317us):
  PE computes G ONCE (fp16 hi/lo 3-term split, fp32 PSUM) -> 3x less PE work.
  The 9-shift sum is assembled hierarchically on the other engines:
    C3 = G + G^(+1,+1) + G^(+2,+2)   (dx level)
    corr = C3 + C3^(+64,+64) + C3^(+128,+128)   (dy level)
  Column(+s) shifts are free AP views. Partition(+s) shifts:
    +1/+2: DMA partition-rebase stages (SDMA, PSUM->SBUF, contiguous rows)
    +64:   ACT partition-rebase copies (probed legal for 64-aligned windows)
    +128:  tile-aligned free view (next C3 tile)
  C3 rows 126,127 of every tile only feed outputs with qx in {62,63}
  (invalid, host-discarded), so stages have NO cross-tile deps: rows 126/127
  are filled with same-tile junk to keep memory initialized.
  corr is materialized compacted over valid p (px<62, py<62 -> 3844 wide);
  max/max_index scan it; host decodes idx via divmod 62.
  Engine balance per output tile: PE 10.3us, DVE ~16.5us (a1 from PSUM,
  corr-add, max, max_index), GPSIMD ~15.7us (a2, t2 adds), ACT ~6.4us
  (stage64), DMA ~12us (stages).

Device strategy (8 NeuronCores, SPMD): shard q: 2 images x 4 blocks of 1024.
Host: unit-normalize, fp16 split, pad; decode idx -> flow + 9 shifts.
"""

import numpy as np

H = W = 64
C = 256
HP = H * W          # 4096 pixel positions per image
PW = 4096           # G/ref pixel columns (exact)
QW_PAD = 4352       # padded input pixel columns for windowing
QBLK = 1024         # q positions per core
QWIN = 1280         # per-core input window width
NQT = 8             # output q-tiles of 128 per core
NCT = 9             # C3/G tiles per core (dy lookahead)
W3 = 4094           # C3 width (cols 0..4093)
NV = 62 * 62        # compact valid-p width (py<62, px<62)

_RUNNER = None


def _build_runner():
    import sys
    if '/opt/trn_rl_repo' not in sys.path:
        sys.path.insert(0, '/opt/trn_rl_repo')
    import concourse.bass as bass  # noqa: F401
    import concourse.tile as tile
    from concourse import bacc, mybir
    from concourse.bass_utils import run_bass_kernel_spmd

    f16 = mybir.dt.float16
    f32 = mybir.dt.float32
    u32 = mybir.dt.uint32

    nc = bacc.Bacc("TRN2", target_bir_lowering=False, debug=False, num_devices=8)

    d_in = {}
    for nm in ("uin_hi", "uin_lo"):
        d_in[nm] = nc.dram_tensor(nm, [C, QWIN], f16, kind="ExternalInput").ap()
    for nm in ("uref_hi", "uref_lo"):
        d_in[nm] = nc.dram_tensor(nm, [C, PW], f16, kind="ExternalInput").ap()
    idx_d = nc.dram_tensor("idx", [128, NQT], u32, kind="ExternalOutput").ap()
    junk_d = nc.dram_tensor("junk", [1, 8], f32, kind="ExternalOutput").ap()

    TERMS = [("uin_hi", "uref_hi"), ("uin_lo", "uref_hi"), ("uin_hi", "uref_lo")]

    def cview(t, py0):
        # [128, 62, 62] view of a [128, 4096] tile: cols 64*(py0+py)+px
        return t[:, 64 * py0: 64 * py0 + 3968].rearrange(
            "p (a b) -> p a b", b=64)[:, :, 0:62]

    with tile.TileContext(nc) as tc:
        with tc.tile_pool(name="const", bufs=1) as cpool, \
             tc.tile_pool(name="work2", bufs=2) as wpool, \
             tc.tile_pool(name="small", bufs=2) as spool, \
             tc.tile_pool(name="stage", bufs=1) as stpool, \
             tc.tile_pool(name="ps", bufs=1, space="PSUM") as ps:

            # Input DMAs; order matters for the wait-absorber below.
            ins = {}
            for nm, w in (("uin_hi", QWIN), ("uref_hi", PW),
                          ("uin_lo", QWIN), ("uref_lo", PW)):
                for ch in range(2):
                    t = cpool.tile([128, w], f16, name="dma_in", tag=f"{nm}{ch}")
                    nc.sync.dma_start(t[:], d_in[nm][128 * ch:128 * (ch + 1), :])
                    ins[(nm, ch)] = t

            # Wait-absorber matmuls: walrus allows only one sync wait on the
            # LDW side of a matmul; absorb the uin-side DMA waits here so real
            # matmuls never carry two fresh DMA waits.
            junk_ps = ps.tile([128, 8], f32, name="junkps", tag="psh0")
            regions = [ins[(nm, ch)]
                       for nm in ("uin_hi", "uin_lo")
                       for ch in range(2)]
            for i, r in enumerate(regions):
                nc.tensor.matmul(junk_ps[:1, :8], r[:, :1], r[:, :8],
                                 start=(i == 0), stop=(i == len(regions) - 1))
            junk_sb = stpool.tile([128, 8], f32, name="junksb", tag="junk")
            nc.vector.tensor_copy(junk_sb[:1, :8], junk_ps[:1, :8])

            idx_stage = stpool.tile([128, NQT], u32, name="idxs", tag="idxs")

            def assemble(t, prev, cur, mx, mi):
                # stage64[q] = C3[q+64, p+64], compact px, ACT rebase copies
                st64 = wpool.tile([128, NV], f32, name="st64", tag="st64")
                nc.scalar.copy(st64[0:64, :], cview(prev, 1)[64:128])
                nc.scalar.copy(st64[64:128, :], cview(cur, 1)[0:64])
                # t2 (GPSIMD, in-place onto st64): t2 = C3[compact] + stage64
                t2 = st64
                nc.gpsimd.tensor_add(t2[:], cview(prev, 0), st64[:])
                # corr (DVE, in-place): t2 += C3_next[py+2 view]
                nc.vector.tensor_add(t2[:], t2[:], cview(cur, 2))
                nc.vector.max(mx[:], t2[:])
                nc.vector.max_index(mi[:], mx[:], t2[:])
                nc.vector.tensor_copy(idx_stage[:, t:t + 1], mi[:, 0:1])

            c3_tiles = {}
            for ct in range(NCT):
                # --- PE: G tile ct in PSUM, two 4-bank halves ---
                halves = []
                for h in range(2):
                    psh = ps.tile([128, 2048], f32, name=f"psh{h}", tag=f"psh{h}")
                    step = 0
                    for (anm, bnm) in TERMS:
                        for ch in range(2):
                            lhsT = ins[(anm, ch)][:, ct * 128: ct * 128 + 128]
                            for bk in range(4):
                                rhs = ins[(bnm, ch)][:, h * 2048 + bk * 512:
                                                     h * 2048 + bk * 512 + 512]
                                nc.tensor.matmul(
                                    psh[:, bk * 512:(bk + 1) * 512], lhsT, rhs,
                                    start=(step == 0), stop=(step == 5))
                            step += 1
                    halves.append(psh)

                # --- ACT: G PSUM -> SBUF (per-bank copies)
                gsb = wpool.tile([128, PW], f32, name="gsb", tag="gsb")
                for h in range(2):
                    for bk in range(4):
                        nc.scalar.copy(
                            gsb[:, h * 2048 + bk * 512: h * 2048 + (bk + 1) * 512],
                            halves[h][:, bk * 512:(bk + 1) * 512])

                # --- shift stages via DMA rebase: s1 = G^(+1,+1), s2 = G^(+2,+2)
                # Rows 126/127 junk (feed only qx in {62,63}, host-discarded).
                s1 = wpool.tile([128, PW], f32, name="s1", tag="s1")
                s2 = wpool.tile([128, PW], f32, name="s2", tag="s2", bufs=1)
                nc.sync.dma_start(s1[0:127, 0:W3], gsb[1:128, 1:1 + W3])
                nc.sync.dma_start(s1[127:128, 0:W3], gsb[127:128, 1:1 + W3])
                nc.scalar.dma_start(s2[0:126, 0:W3], gsb[2:128, 2:2 + W3])
                nc.scalar.dma_start(s2[126:128, 0:W3], gsb[126:128, 2:2 + W3])

                # --- a1 (GPSIMD): t1 = G + s1
                c3 = wpool.tile([128, PW], f32, name="c3", tag="c3", bufs=3)
                nc.gpsimd.tensor_add(c3[:, 0:W3], gsb[:, 0:W3], s1[:, 0:W3])
                # --- a2 (DVE, in-place): c3 += s2
                nc.vector.tensor_add(c3[:, 0:W3], c3[:, 0:W3], s2[:, 0:W3])
                c3_tiles[ct] = c3
                if ct < 2:
                    continue

                # 2-tile-lag assembly: everything it reads is already done,
                # so no engine queue ever head-of-line blocks on a future dep.
                t = ct - 2
                mx = spool.tile([128, 8], f32, name="mx", tag="mx")
                mi = spool.tile([128, 8], u32, name="mi", tag="mi")
                assemble(t, c3_tiles[t], c3_tiles[t + 1], mx, mi)
                del c3_tiles[t]

            mx = spool.tile([128, 8], f32, name="mxf", tag="mx")
            mi = spool.tile([128, 8], u32, name="mif", tag="mi")
            assemble(NCT - 2, c3_tiles[NCT - 2], c3_tiles[NCT - 1], mx, mi)

            nc.sync.dma_start(idx_d[:], idx_stage[:])
            nc.sync.dma_start(junk_d[:], junk_sb[:1, :8])

    nc.compile()
    return nc, run_bass_kernel_spmd


def _unit_pixels(f):
    # f: (C, H, W) float32; unit L2 norm per pixel across channels (fp32 math)
    n = np.sqrt(np.sum(f * f, axis=0, keepdims=True, dtype=np.float32))
    return (f / np.maximum(n, np.float32(1e-12))).astype(np.float32)


def _split_f16(a):
    hi = a.astype(np.float16)
    lo = (a - hi.astype(np.float32)).astype(np.float16)
    return hi, lo


def kernel(dense_features1, dense_features2, img_ref_hr):
    global _RUNNER
    if _RUNNER is None:
        _RUNNER = _build_runner()
    nc, run_spmd = _RUNNER

    f1 = np.asarray(dense_features1, dtype=np.float32)  # input features (b,C,H,W)
    f2 = np.asarray(dense_features2, dtype=np.float32)  # ref features
    B = f1.shape[0]
    assert B == 2 and f1.shape[1:] == (C, H, W)

    in_maps = []
    per_img = []
    for b in range(B):
        fin_u = _unit_pixels(f1[b]).reshape(C, HP)
        fref_u = _unit_pixels(f2[b]).reshape(C, HP)
        uin = np.zeros((C, QW_PAD), np.float32)
        uin[:, :HP] = fin_u
        uref = fref_u
        uin_hi, uin_lo = _split_f16(uin)
        uref_hi, uref_lo = _split_f16(uref)
        per_img.append((uin_hi, uin_lo, uref_hi, uref_lo))

    for core in range(8):
        b, qblk = divmod(core, 4)
        uin_hi, uin_lo, uref_hi, uref_lo = per_img[b]
        q0 = qblk * QBLK
        in_maps.append({
            "uin_hi": np.ascontiguousarray(uin_hi[:, q0:q0 + QWIN]),
            "uin_lo": np.ascontiguousarray(uin_lo[:, q0:q0 + QWIN]),
            "uref_hi": np.ascontiguousarray(uref_hi),
            "uref_lo": np.ascontiguousarray(uref_lo),
        })

    results = run_spmd(nc, in_maps, list(range(8))).results

    # Decode: idx_stage[part, tile] = argmax over compact (py, px) grid for
    # q_local = tile*128 + part, global q = core_q0 + q_local.
    out = np.zeros((B, 9, H, W, 2), np.float32)
    qx_grid = np.arange(62, dtype=np.float32)[None, :]
    qy_grid = np.arange(62, dtype=np.float32)[:, None]
    for b in range(B):
        idx_full = np.zeros(HP, np.int64)
        for qblk in range(4):
            r = results[b * 4 + qblk]["idx"]  # (128, NQT) uint32
            idx_full[qblk * QBLK:(qblk + 1) * QBLK] = r.T.reshape(-1)
        idx_grid = idx_full.reshape(H, W)[:62, :62]
        py = (idx_grid // 62).astype(np.float32)
        px = (idx_grid % 62).astype(np.float32)
        flow = np.zeros((H, W, 2), np.float32)
        flow[:62, :62, 0] = px - qx_grid
        flow[:62, :62, 1] = py - qy_grid
        for k, (i, j) in enumerate([(i, j) for i in range(3) for j in range(3)]):
            out[b, k, i:, j:, :] = flow[:H - i, :W - j, :]
    return out


# revision 3
# speedup vs baseline: 1.2068x; 1.0323x over previous
"""Trainium2 Bass kernel for CorrespondenceFeatGeneration (patch-correlation argmax flow).

Math (per image, mirrors the reference):
  fin, fref: (256, 64, 64) -> unit-normalize each pixel across channels.
  corr[q, p] = <3x3 patch of fref at p, 3x3 patch of fin at q>   (2304-dim dot)
             = sum_{s in {0,1,2,64,65,66,128,129,130}} G[q+s, p+s],
               G[a, b] = <u_a, v_b>  (pixel correlation, K=256)
  max_idx[q] = argmax_{valid p} corr[q, p]  (first-max tie-break)

v2 structure (vs the dx-folded-into-PE baseline at ~317us):
  PE computes G ONCE (fp16 hi/lo 3-term split, fp32 PSUM) -> 3x less PE work.
  The 9-shift sum is assembled hierarchically on the other engines:
    C3 = G + G^(+1,+1) + G^(+2,+2)   (dx level)
    corr = C3 + C3^(+64,+64) + C3^(+128,+128)   (dy level)
  Column(+s) shifts are free AP views. Partition(+s) shifts:
    +1/+2: DMA partition-rebase stages (SDMA, PSUM->SBUF, contiguous rows)
    +64:   ACT partition-rebase copies (probed legal for 64-aligned windows)
    +128:  tile-aligned free view (next C3 tile)
  C3 rows 126,127 of every tile only feed outputs with qx in {62,63}
  (invalid, host-discarded), so stages have NO cross-tile deps: rows 126/127
  are filled with same-tile junk to keep memory initialized.
  corr is materialized compacted over valid p (px<62, py<62 -> 3844 wide);
  max/max_index scan it; host decodes idx via divmod 62.
  Engine balance per output tile: PE 10.3us, DVE ~16.5us (a1 from PSUM,
  corr-add, max, max_index), GPSIMD ~15.7us (a2, t2 adds), ACT ~6.4us
  (stage64), DMA ~12us (stages).

Device strategy (8 NeuronCores, SPMD): shard q: 2 images x 4 blocks of 1024.
Host: unit-normalize, fp16 split, pad; decode idx -> flow + 9 shifts.
"""

import numpy as np

H = W = 64
C = 256
HP = H * W          # 4096 pixel positions per image
PW = 4096           # G/ref pixel columns (exact)
QW_PAD = 4352       # padded input pixel columns for windowing
QBLK = 1024         # q positions per core
QWIN = 1280         # per-core input window width
NQT = 8             # output q-tiles of 128 per core
NCT = 9             # C3/G tiles per core (dy lookahead)
W3 = 4094           # C3 width (cols 0..4093)
NV = 62 * 62        # compact valid-p width (py<62, px<62)

_RUNNER = None


def _build_runner():
    import sys
    if '/opt/trn_rl_repo' not in sys.path:
        sys.path.insert(0, '/opt/trn_rl_repo')
    import concourse.bass as bass  # noqa: F401
    import concourse.tile as tile
    from concourse import bacc, mybir
    from concourse.bass_utils import run_bass_kernel_spmd

    f16 = mybir.dt.float16
    f32 = mybir.dt.float32
    u32 = mybir.dt.uint32

    nc = bacc.Bacc("TRN2", target_bir_lowering=False, debug=False, num_devices=8)

    d_in = {}
    for nm in ("uin_hi", "uin_lo"):
        d_in[nm] = nc.dram_tensor(nm, [C, QWIN], f16, kind="ExternalInput").ap()
    for nm in ("uref_hi", "uref_lo"):
        d_in[nm] = nc.dram_tensor(nm, [C, PW], f16, kind="ExternalInput").ap()
    idx_d = nc.dram_tensor("idx", [128, NQT], u32, kind="ExternalOutput").ap()
    junk_d = nc.dram_tensor("junk", [1, 8], f32, kind="ExternalOutput").ap()

    TERMS = [("uin_hi", "uref_hi"), ("uin_lo", "uref_hi"), ("uin_hi", "uref_lo")]

    def cview(t, py0):
        # [128, 62, 62] view of a [128, 4096] tile: cols 64*(py0+py)+px
        return t[:, 64 * py0: 64 * py0 + 3968].rearrange(
            "p (a b) -> p a b", b=64)[:, :, 0:62]

    with tile.TileContext(nc) as tc:
        with tc.tile_pool(name="const", bufs=1) as cpool, \
             tc.tile_pool(name="work2", bufs=2) as wpool, \
             tc.tile_pool(name="small", bufs=2) as spool, \
             tc.tile_pool(name="stage", bufs=1) as stpool, \
             tc.tile_pool(name="ps", bufs=1, space="PSUM") as ps:

            # Input DMAs; order matters for the wait-absorber below.
            ins = {}
            for nm, w in (("uin_hi", QWIN), ("uref_hi", PW),
                          ("uin_lo", QWIN), ("uref_lo", PW)):
                for ch in range(2):
                    t = cpool.tile([128, w], f16, name="dma_in", tag=f"{nm}{ch}")
                    nc.sync.dma_start(t[:], d_in[nm][128 * ch:128 * (ch + 1), :])
                    ins[(nm, ch)] = t

            # Wait-absorber matmuls: walrus allows only one sync wait on the
            # LDW side of a matmul; absorb the uin-side DMA waits here so real
            # matmuls never carry two fresh DMA waits.
            junk_ps = ps.tile([128, 8], f32, name="junkps", tag="pspc0")
            regions = [ins[(nm, ch)]
                       for nm in ("uin_hi", "uin_lo")
                       for ch in range(2)]
            for i, r in enumerate(regions):
                nc.tensor.matmul(junk_ps[:1, :8], r[:, :1], r[:, :8],
                                 start=(i == 0), stop=(i == len(regions) - 1))
            junk_sb = stpool.tile([128, 8], f32, name="junksb", tag="junk")
            nc.vector.tensor_copy(junk_sb[:1, :8], junk_ps[:1, :8])

            idx_stage = stpool.tile([128, NQT], u32, name="idxs", tag="idxs")

            def assemble(t, prev, cur, mx, mi):
                # stage64[q] = C3[q+64, p+64], compact px, ACT rebase copies
                st64 = wpool.tile([128, NV], f32, name="st64", tag="st64")
                nc.scalar.copy(st64[0:64, :], cview(prev, 1)[64:128])
                nc.scalar.copy(st64[64:128, :], cview(cur, 1)[0:64])
                # t2 (GPSIMD, in-place onto st64): t2 = C3[compact] + stage64
                t2 = st64
                nc.gpsimd.tensor_add(t2[:], cview(prev, 0), st64[:])
                # corr (DVE, in-place): t2 += C3_next[py+2 view]
                nc.vector.tensor_add(t2[:], t2[:], cview(cur, 2))
                nc.vector.max(mx[:], t2[:])
                nc.vector.max_index(mi[:], mx[:], t2[:])
                nc.vector.tensor_copy(idx_stage[:, t:t + 1], mi[:, 0:1])

            c3_tiles = {}
            for ct in range(NCT):
                # --- PE: G tile ct in PSUM, four 2-bank pieces; ACT copies
                # each piece to SBUF right after its matmuls so the next
                # tile's PE piece is never blocked long (keeps PE p-state hot).
                gsb = wpool.tile([128, PW], f32, name="gsb", tag="gsb")
                for pc in range(4):
                    psp = ps.tile([128, 1024], f32, name=f"pspc{pc}",
                                  tag=f"pspc{pc}")
                    step = 0
                    for (anm, bnm) in TERMS:
                        for ch in range(2):
                            lhsT = ins[(anm, ch)][:, ct * 128: ct * 128 + 128]
                            for bk in range(2):
                                rhs = ins[(bnm, ch)][:, pc * 1024 + bk * 512:
                                                     pc * 1024 + bk * 512 + 512]
                                nc.tensor.matmul(
                                    psp[:, bk * 512:(bk + 1) * 512], lhsT, rhs,
                                    start=(step == 0), stop=(step == 5))
                            step += 1
                    for bk in range(2):
                        nc.scalar.copy(
                            gsb[:, pc * 1024 + bk * 512: pc * 1024 + (bk + 1) * 512],
                            psp[:, bk * 512:(bk + 1) * 512])

                # --- shift stages via DMA rebase: s1 = G^(+1,+1), s2 = G^(+2,+2)
                # Rows 126/127 junk (feed only qx in {62,63}, host-discarded).
                s1 = wpool.tile([128, PW], f32, name="s1", tag="s1")
                s2 = wpool.tile([128, PW], f32, name="s2", tag="s2", bufs=1)
                nc.sync.dma_start(s1[0:127, 0:W3], gsb[1:128, 1:1 + W3])
                nc.sync.dma_start(s1[127:128, 0:W3], gsb[127:128, 1:1 + W3])
                nc.scalar.dma_start(s2[0:126, 0:W3], gsb[2:128, 2:2 + W3])
                nc.scalar.dma_start(s2[126:128, 0:W3], gsb[126:128, 2:2 + W3])

                # --- a1 (GPSIMD): t1 = G + s1
                c3 = wpool.tile([128, PW], f32, name="c3", tag="c3", bufs=3)
                nc.gpsimd.tensor_add(c3[:, 0:W3], gsb[:, 0:W3], s1[:, 0:W3])
                # --- a2 (DVE, in-place): c3 += s2
                nc.vector.tensor_add(c3[:, 0:W3], c3[:, 0:W3], s2[:, 0:W3])
                c3_tiles[ct] = c3
                if ct < 2:
                    continue

                # 2-tile-lag assembly: everything it reads is already done,
                # so no engine queue ever head-of-line blocks on a future dep.
                t = ct - 2
                mx = spool.tile([128, 8], f32, name="mx", tag="mx")
                mi = spool.tile([128, 8], u32, name="mi", tag="mi")
                assemble(t, c3_tiles[t], c3_tiles[t + 1], mx, mi)
                del c3_tiles[t]

            mx = spool.tile([128, 8], f32, name="mxf", tag="mx")
            mi = spool.tile([128, 8], u32, name="mif", tag="mi")
            assemble(NCT - 2, c3_tiles[NCT - 2], c3_tiles[NCT - 1], mx, mi)

            nc.sync.dma_start(idx_d[:], idx_stage[:])
            nc.sync.dma_start(junk_d[:], junk_sb[:1, :8])

    nc.compile()
    return nc, run_bass_kernel_spmd


def _unit_pixels(f):
    # f: (C, H, W) float32; unit L2 norm per pixel across channels (fp32 math)
    n = np.sqrt(np.sum(f * f, axis=0, keepdims=True, dtype=np.float32))
    return (f / np.maximum(n, np.float32(1e-12))).astype(np.float32)


def _split_f16(a):
    hi = a.astype(np.float16)
    lo = (a - hi.astype(np.float32)).astype(np.float16)
    return hi, lo


def kernel(dense_features1, dense_features2, img_ref_hr):
    global _RUNNER
    if _RUNNER is None:
        _RUNNER = _build_runner()
    nc, run_spmd = _RUNNER

    f1 = np.asarray(dense_features1, dtype=np.float32)  # input features (b,C,H,W)
    f2 = np.asarray(dense_features2, dtype=np.float32)  # ref features
    B = f1.shape[0]
    assert B == 2 and f1.shape[1:] == (C, H, W)

    in_maps = []
    per_img = []
    for b in range(B):
        fin_u = _unit_pixels(f1[b]).reshape(C, HP)
        fref_u = _unit_pixels(f2[b]).reshape(C, HP)
        uin = np.zeros((C, QW_PAD), np.float32)
        uin[:, :HP] = fin_u
        uref = fref_u
        uin_hi, uin_lo = _split_f16(uin)
        uref_hi, uref_lo = _split_f16(uref)
        per_img.append((uin_hi, uin_lo, uref_hi, uref_lo))

    for core in range(8):
        b, qblk = divmod(core, 4)
        uin_hi, uin_lo, uref_hi, uref_lo = per_img[b]
        q0 = qblk * QBLK
        in_maps.append({
            "uin_hi": np.ascontiguousarray(uin_hi[:, q0:q0 + QWIN]),
            "uin_lo": np.ascontiguousarray(uin_lo[:, q0:q0 + QWIN]),
            "uref_hi": np.ascontiguousarray(uref_hi),
            "uref_lo": np.ascontiguousarray(uref_lo),
        })

    results = run_spmd(nc, in_maps, list(range(8))).results

    # Decode: idx_stage[part, tile] = argmax over compact (py, px) grid for
    # q_local = tile*128 + part, global q = core_q0 + q_local.
    out = np.zeros((B, 9, H, W, 2), np.float32)
    qx_grid = np.arange(62, dtype=np.float32)[None, :]
    qy_grid = np.arange(62, dtype=np.float32)[:, None]
    for b in range(B):
        idx_full = np.zeros(HP, np.int64)
        for qblk in range(4):
            r = results[b * 4 + qblk]["idx"]  # (128, NQT) uint32
            idx_full[qblk * QBLK:(qblk + 1) * QBLK] = r.T.reshape(-1)
        idx_grid = idx_full.reshape(H, W)[:62, :62]
        py = (idx_grid // 62).astype(np.float32)
        px = (idx_grid % 62).astype(np.float32)
        flow = np.zeros((H, W, 2), np.float32)
        flow[:62, :62, 0] = px - qx_grid
        flow[:62, :62, 1] = py - qy_grid
        for k, (i, j) in enumerate([(i, j) for i in range(3) for j in range(3)]):
            out[b, k, i:, j:, :] = flow[:H - i, :W - j, :]
    return out


# revision 4
# speedup vs baseline: 1.2163x; 1.0079x over previous
"""Trainium2 Bass kernel for CorrespondenceFeatGeneration (patch-correlation argmax flow).

Math (per image, mirrors the reference):
  fin, fref: (256, 64, 64) -> unit-normalize each pixel across channels.
  corr[q, p] = <3x3 patch of fref at p, 3x3 patch of fin at q>   (2304-dim dot)
             = sum_{s in {0,1,2,64,65,66,128,129,130}} G[q+s, p+s],
               G[a, b] = <u_a, v_b>  (pixel correlation, K=256)
  max_idx[q] = argmax_{valid p} corr[q, p]  (first-max tie-break)

v2 structure (vs the dx-folded-into-PE baseline at ~317us; TimelineSim
makespan ~194us, verified bit-exact on HW):
  PE computes G ONCE (fp16 hi/lo 3-term split, fp32 PSUM) -> 3x less PE work.
  The 9-shift sum is assembled hierarchically on the other engines:
    C3 = G + G^(+1,+1) + G^(+2,+2)   (dx level)
    corr = C3 + C3^(+64,+64) + C3^(+128,+128)   (dy level)
  Column(+s) shifts are free AP views. Partition(+s) shifts:
    +1/+2: SDMA partition-rebase stages (SBUF->SBUF, contiguous rows,
           split L/R at the PSUM-piece boundary for early starts)
    +64:   ACT partition-rebase copies (HW-probed legal for 64-aligned
           windows; engines CANNOT rebase by non-32-aligned offsets, and
           tensor_tensor requires equal input partition bases -- probed)
    +128:  tile-aligned free view (next C3 tile)
  C3 rows 126,127 of every tile only feed outputs with qx in {62,63}
  (invalid, host-discarded), so the shift stages have NO cross-tile deps:
  rows 126/127 are junk-filled from same-tile rows to stay finite.
  corr is materialized px-compacted over valid p (px<62, py<62 -> 3844
  wide); max/max_index scan it; host decodes idx via divmod 62.
  Pipeline notes (why it hits ~17us/tile steady state, DVE+Pool ~98% busy):
   - G in 4 PSUM pieces (2 banks each); ACT copies each piece to SBUF right
     after its 12 matmuls so the PE almost never stalls (p-state stays hot).
   - assembly runs with a 2-TILE LAG so no engine FIFO ever head-of-line
     blocks on a just-produced c3 (every input is >=1 iteration old).
   - s2 stage DMAs issue on the ACT HWDGE ring (nc.scalar) so their waits
     do not block the SP ring; s2/st64 gate on events that are past anyway.
   - engine split per tile: Pool: a1 L/R + t2-part; DVE: a2 L/R, corr-part,
     max, max_index; ACT: 8 psum copies + stage64; DMA: 4 stage + 4 junk.
  tensor_tensor_reduce (fused add+max) hard-crashes the device in this
  toolchain -- do not use. SWDGE (nc.gpsimd.dma_start) silently no-ops
  under the axon/PJRT path -- do not use.

Device strategy (8 NeuronCores, SPMD): shard q: 2 images x 4 blocks of 1024.
Host: unit-normalize, fp16 split, pad; decode idx -> flow + 9 shifts.
"""

import numpy as np

H = W = 64
C = 256
HP = H * W          # 4096 pixel positions per image
PW = 4096           # G/ref pixel columns (exact)
QW_PAD = 4352       # padded input pixel columns for windowing
QBLK = 1024         # q positions per core
QWIN = 1280         # per-core input window width
NQT = 8             # output q-tiles of 128 per core
NCT = 9             # C3/G tiles per core (dy lookahead)
W3 = 4094           # C3 width (cols 0..4093)
NV = 62 * 62        # compact valid-p width (py<62, px<62)

_RUNNER = None


def _build_runner():
    import sys
    if '/opt/trn_rl_repo' not in sys.path:
        sys.path.insert(0, '/opt/trn_rl_repo')
    import concourse.bass as bass  # noqa: F401
    import concourse.tile as tile
    from concourse import bacc, mybir
    from concourse.bass_utils import run_bass_kernel_spmd

    f16 = mybir.dt.float16
    f32 = mybir.dt.float32
    u32 = mybir.dt.uint32

    nc = bacc.Bacc("TRN2", target_bir_lowering=False, debug=False, num_devices=8)

    d_in = {}
    for nm in ("uin_hi", "uin_lo"):
        d_in[nm] = nc.dram_tensor(nm, [C, QWIN], f16, kind="ExternalInput").ap()
    for nm in ("uref_hi", "uref_lo"):
        d_in[nm] = nc.dram_tensor(nm, [C, PW], f16, kind="ExternalInput").ap()
    idx_d = nc.dram_tensor("idx", [128, NQT], u32, kind="ExternalOutput").ap()
    junk_d = nc.dram_tensor("junk", [1, 8], f32, kind="ExternalOutput").ap()

    TERMS = [("uin_hi", "uref_hi"), ("uin_lo", "uref_hi"), ("uin_hi", "uref_lo")]

    def cview(t, py0):
        # [128, 62, 62] view of a [128, 4096] tile: cols 64*(py0+py)+px
        return t[:, 64 * py0: 64 * py0 + 3968].rearrange(
            "p (a b) -> p a b", b=64)[:, :, 0:62]

    with tile.TileContext(nc) as tc:
        with tc.tile_pool(name="const", bufs=1) as cpool, \
             tc.tile_pool(name="work2", bufs=2) as wpool, \
             tc.tile_pool(name="small", bufs=2) as spool, \
             tc.tile_pool(name="stage", bufs=1) as stpool, \
             tc.tile_pool(name="ps", bufs=1, space="PSUM") as ps:

            # Input DMAs; order matters for the wait-absorber below.
            # uref loads split by column half so early PE pieces start sooner.
            ins = {}
            for nm, w in (("uin_hi", QWIN), ("uref_hi", PW),
                          ("uin_lo", QWIN), ("uref_lo", PW)):
                for ch in range(2):
                    t = cpool.tile([128, w], f16, name="dma_in", tag=f"{nm}{ch}")
                    ins[(nm, ch)] = t
                    if w == QWIN:
                        nc.sync.dma_start(t[:], d_in[nm][128 * ch:128 * (ch + 1), :])
                    else:
                        nc.sync.dma_start(t[:, 0:2048],
                                          d_in[nm][128 * ch:128 * (ch + 1), 0:2048])
                        nc.sync.dma_start(t[:, 2048:PW],
                                          d_in[nm][128 * ch:128 * (ch + 1), 2048:PW])

            # Wait-absorber matmuls: walrus allows only one sync wait on the
            # LDW side of a matmul; absorb the uin-side DMA waits here so real
            # matmuls never carry two fresh DMA waits.
            junk_ps = ps.tile([128, 8], f32, name="junkps", tag="pspc3")
            regions = [ins[(nm, ch)]
                       for nm in ("uin_hi", "uin_lo")
                       for ch in range(2)]
            for i, r in enumerate(regions):
                nc.tensor.matmul(junk_ps[:1, :8], r[:, :1], r[:, :8],
                                 start=(i == 0), stop=(i == len(regions) - 1))
            junk_sb = stpool.tile([128, 8], f32, name="junksb", tag="junk")
            nc.vector.tensor_copy(junk_sb[:1, :8], junk_ps[:1, :8])

            idx_stage = stpool.tile([128, NQT], u32, name="idxs", tag="idxs")

            def assemble(t, prev, cur, mx, mi):
                # stage64[q] = C3[q+64, p+64], compact px, ACT rebase copies
                st64 = wpool.tile([128, NV], f32, name="st64", tag="st64")
                nc.scalar.copy(st64[0:64, :], cview(prev, 1)[64:128])
                nc.scalar.copy(st64[64:128, :], cview(cur, 1)[0:64])
                # t2 = C3[compact] + stage64 (in-place onto st64);
                # corr: t2 += C3_next[py+2 view]. Pool/DVE work concurrently.
                t2 = st64
                SP = 40 * 62  # 40 of 62 py-groups on Pool for t2
                nc.gpsimd.tensor_add(t2[:, 0:SP], cview(prev, 0)[:, 0:40, :],
                                     st64[:, 0:SP])
                nc.vector.tensor_add(t2[:, SP:], cview(prev, 0)[:, 40:62, :],
                                     st64[:, SP:])
                nc.vector.tensor_add(t2[:, 0:SP], t2[:, 0:SP],
                                     cview(cur, 2)[:, 0:40, :])
                nc.gpsimd.tensor_add(t2[:, SP:], t2[:, SP:],
                                     cview(cur, 2)[:, 40:62, :])
                nc.vector.max(mx[:], t2[:])
                nc.vector.max_index(mi[:], mx[:], t2[:])
                nc.vector.tensor_copy(idx_stage[:, t:t + 1], mi[:, 0:1])

            c3_tiles = {}
            for ct in range(NCT):
                # --- PE: G tile ct in PSUM, four 2-bank pieces; ACT copies
                # each piece to SBUF right after its matmuls so the next
                # tile's PE piece is never blocked long (keeps PE p-state hot).
                gsb = wpool.tile([128, PW], f32, name="gsb", tag="gsb")
                for pc in range(4):
                    psp = ps.tile([128, 1024], f32, name=f"pspc{pc}",
                                  tag=f"pspc{pc}")
                    step = 0
                    for (anm, bnm) in TERMS:
                        for ch in range(2):
                            lhsT = ins[(anm, ch)][:, ct * 128: ct * 128 + 128]
                            for bk in range(2):
                                rhs = ins[(bnm, ch)][:, pc * 1024 + bk * 512:
                                                     pc * 1024 + bk * 512 + 512]
                                nc.tensor.matmul(
                                    psp[:, bk * 512:(bk + 1) * 512], lhsT, rhs,
                                    start=(step == 0), stop=(step == 5))
                            step += 1
                    for bk in range(2):
                        nc.scalar.copy(
                            gsb[:, pc * 1024 + bk * 512: pc * 1024 + (bk + 1) * 512],
                            psp[:, bk * 512:(bk + 1) * 512])

                # --- shift stages via DMA rebase, split L/R at the gsb
                # piece boundary so each half starts as soon as its source
                # pieces land: s1 = G^(+1,+1), s2 = G^(+2,+2).
                # Rows 126/127 junk (feed only qx in {62,63}, host-discarded).
                s1L = wpool.tile([128, 2047], f32, name="s1L", tag="s1L")
                s1R = wpool.tile([128, 2047], f32, name="s1R", tag="s1R")
                s2L = wpool.tile([128, 2046], f32, name="s2L", tag="s2L", bufs=1)
                s2R = wpool.tile([128, 2048], f32, name="s2R", tag="s2R", bufs=1)
                nc.sync.dma_start(s1L[0:127, :], gsb[1:128, 1:2048])
                nc.sync.dma_start(s1L[127:128, :], gsb[127:128, 1:2048])
                nc.scalar.dma_start(s2L[0:126, :], gsb[2:128, 2:2048])
                nc.scalar.dma_start(s2L[126:128, :], gsb[126:128, 2:2048])
                nc.sync.dma_start(s1R[0:127, :], gsb[1:128, 2048:W3 + 1])
                nc.sync.dma_start(s1R[127:128, :], gsb[127:128, 2048:W3 + 1])
                nc.scalar.dma_start(s2R[0:126, :], gsb[2:128, 2048:W3 + 2])
                nc.scalar.dma_start(s2R[126:128, :], gsb[126:128, 2048:W3 + 2])

                # --- a1 (GPSIMD) L/R: c3 = G + s1;  a2 (DVE, in-place) += s2
                c3 = wpool.tile([128, PW], f32, name="c3", tag="c3", bufs=3)
                nc.gpsimd.tensor_add(c3[:, 0:2047], gsb[:, 0:2047], s1L[:])
                nc.vector.tensor_add(c3[:, 0:2046], c3[:, 0:2046], s2L[:])
                nc.gpsimd.tensor_add(c3[:, 2047:W3], gsb[:, 2047:W3], s1R[:])
                nc.vector.tensor_add(c3[:, 2046:W3], c3[:, 2046:W3], s2R[:])
                c3_tiles[ct] = c3
                if ct < 2:
                    continue

                # 2-tile-lag assembly: everything it reads is already done,
                # so no engine queue ever head-of-line blocks on a future dep.
                t = ct - 2
                mx = spool.tile([128, 8], f32, name="mx", tag="mx")
                mi = spool.tile([128, 8], u32, name="mi", tag="mi")
                assemble(t, c3_tiles[t], c3_tiles[t + 1], mx, mi)
                del c3_tiles[t]

            mx = spool.tile([128, 8], f32, name="mxf", tag="mx")
            mi = spool.tile([128, 8], u32, name="mif", tag="mi")
            assemble(NCT - 2, c3_tiles[NCT - 2], c3_tiles[NCT - 1], mx, mi)

            nc.sync.dma_start(idx_d[:], idx_stage[:])
            nc.sync.dma_start(junk_d[:], junk_sb[:1, :8])

    nc.compile()
    return nc, run_bass_kernel_spmd


def _unit_pixels(f):
    # f: (C, H, W) float32; unit L2 norm per pixel across channels (fp32 math)
    n = np.sqrt(np.sum(f * f, axis=0, keepdims=True, dtype=np.float32))
    return (f / np.maximum(n, np.float32(1e-12))).astype(np.float32)


def _split_f16(a):
    hi = a.astype(np.float16)
    lo = (a - hi.astype(np.float32)).astype(np.float16)
    return hi, lo


def kernel(dense_features1, dense_features2, img_ref_hr):
    global _RUNNER
    if _RUNNER is None:
        _RUNNER = _build_runner()
    nc, run_spmd = _RUNNER

    f1 = np.asarray(dense_features1, dtype=np.float32)  # input features (b,C,H,W)
    f2 = np.asarray(dense_features2, dtype=np.float32)  # ref features
    B = f1.shape[0]
    assert B == 2 and f1.shape[1:] == (C, H, W)

    in_maps = []
    per_img = []
    for b in range(B):
        fin_u = _unit_pixels(f1[b]).reshape(C, HP)
        fref_u = _unit_pixels(f2[b]).reshape(C, HP)
        uin = np.zeros((C, QW_PAD), np.float32)
        uin[:, :HP] = fin_u
        uref = fref_u
        uin_hi, uin_lo = _split_f16(uin)
        uref_hi, uref_lo = _split_f16(uref)
        per_img.append((uin_hi, uin_lo, uref_hi, uref_lo))

    for core in range(8):
        b, qblk = divmod(core, 4)
        uin_hi, uin_lo, uref_hi, uref_lo = per_img[b]
        q0 = qblk * QBLK
        in_maps.append({
            "uin_hi": np.ascontiguousarray(uin_hi[:, q0:q0 + QWIN]),
            "uin_lo": np.ascontiguousarray(uin_lo[:, q0:q0 + QWIN]),
            "uref_hi": np.ascontiguousarray(uref_hi),
            "uref_lo": np.ascontiguousarray(uref_lo),
        })

    results = run_spmd(nc, in_maps, list(range(8))).results

    # Decode: idx_stage[part, tile] = argmax over compact (py, px) grid for
    # q_local = tile*128 + part, global q = core_q0 + q_local.
    out = np.zeros((B, 9, H, W, 2), np.float32)
    qx_grid = np.arange(62, dtype=np.float32)[None, :]
    qy_grid = np.arange(62, dtype=np.float32)[:, None]
    for b in range(B):
        idx_full = np.zeros(HP, np.int64)
        for qblk in range(4):
            r = results[b * 4 + qblk]["idx"]  # (128, NQT) uint32
            idx_full[qblk * QBLK:(qblk + 1) * QBLK] = r.T.reshape(-1)
        idx_grid = idx_full.reshape(H, W)[:62, :62]
        py = (idx_grid // 62).astype(np.float32)
        px = (idx_grid % 62).astype(np.float32)
        flow = np.zeros((H, W, 2), np.float32)
        flow[:62, :62, 0] = px - qx_grid
        flow[:62, :62, 1] = py - qy_grid
        for k, (i, j) in enumerate([(i, j) for i in range(3) for j in range(3)]):
            out[b, k, i:, j:, :] = flow[:H - i, :W - j, :]
    return out


# revision 5
# speedup vs baseline: 1.2201x; 1.0031x over previous
"""Trainium2 Bass kernel for CorrespondenceFeatGeneration (patch-correlation argmax flow).

Math (per image, mirrors the reference):
  fin, fref: (256, 64, 64) -> unit-normalize each pixel across channels.
  corr[q, p] = <3x3 patch of fref at p, 3x3 patch of fin at q>   (2304-dim dot)
             = sum_{s in {0,1,2,64,65,66,128,129,130}} G[q+s, p+s],
               G[a, b] = <u_a, v_b>  (pixel correlation, K=256)
  max_idx[q] = argmax_{valid p} corr[q, p]  (first-max tie-break)

v2 structure (vs the dx-folded-into-PE baseline at ~317us; TimelineSim
makespan ~191us, verified bit-exact on HW):
  PE computes G ONCE (fp16 hi/lo 3-term split, fp32 PSUM) -> 3x less PE work.
  The 9-shift sum is assembled hierarchically on the other engines:
    C3 = G + G^(+1,+1) + G^(+2,+2)   (dx level)
    corr = C3 + C3^(+64,+64) + C3^(+128,+128)   (dy level)
  Column(+s) shifts are free AP views. Partition(+s) shifts:
    +1/+2: SDMA partition-rebase stages (SBUF->SBUF, contiguous rows,
           split L/R at the PSUM-piece boundary for early starts)
    +64:   ACT partition-rebase copies (HW-probed legal for 64-aligned
           windows; engines CANNOT rebase by non-32-aligned offsets, and
           tensor_tensor requires equal input partition bases -- probed)
    +128:  tile-aligned free view (next C3 tile)
  C3 rows 126,127 of every tile only feed outputs with qx in {62,63}
  (invalid, host-discarded), so the shift stages have NO cross-tile deps:
  rows 126/127 are junk-filled from same-tile rows to stay finite.
  corr is materialized px-compacted over valid p (px<62, py<62 -> 3844
  wide); max/max_index scan it; host decodes idx via divmod 62.
  Pipeline notes (why it hits ~17us/tile steady state, DVE+Pool ~98% busy):
   - G in 4 PSUM pieces (2 banks each); ACT copies each piece to SBUF right
     after its 12 matmuls so the PE almost never stalls (p-state stays hot).
   - assembly runs with a 2-TILE LAG so no engine FIFO ever head-of-line
     blocks on a just-produced c3 (every input is >=1 iteration old).
   - s2 stage DMAs issue on the ACT HWDGE ring (nc.scalar) so their waits
     do not block the SP ring; s2/st64 gate on events that are past anyway.
   - engine split per tile: Pool: a1 L/R + t2-part; DVE: a2 L/R, corr-part,
     max, max_index; ACT: 8 psum copies + stage64; DMA: 4 stage + 4 junk.
  tensor_tensor_reduce (fused add+max) hard-crashes the device in this
  toolchain -- do not use. SWDGE (nc.gpsimd.dma_start) silently no-ops
  under the axon/PJRT path -- do not use.

Device strategy (8 NeuronCores, SPMD): shard q: 2 images x 4 blocks of 1024.
Host: unit-normalize, fp16 split, pad; decode idx -> flow + 9 shifts.
"""

import numpy as np

H = W = 64
C = 256
HP = H * W          # 4096 pixel positions per image
PW = 4096           # G/ref pixel columns (exact)
QW_PAD = 4352       # padded input pixel columns for windowing
QBLK = 1024         # q positions per core
QWIN = 1280         # per-core input window width
NQT = 8             # output q-tiles of 128 per core
NCT = 9             # C3/G tiles per core (dy lookahead)
W3 = 4094           # C3 width (cols 0..4093)
NV = 62 * 62        # compact valid-p width (py<62, px<62)

_RUNNER = None


def _build_runner():
    import sys
    if '/opt/trn_rl_repo' not in sys.path:
        sys.path.insert(0, '/opt/trn_rl_repo')
    import concourse.bass as bass  # noqa: F401
    import concourse.tile as tile
    from concourse import bacc, mybir
    from concourse.bass_utils import run_bass_kernel_spmd

    f16 = mybir.dt.float16
    f32 = mybir.dt.float32
    u32 = mybir.dt.uint32

    nc = bacc.Bacc("TRN2", target_bir_lowering=False, debug=False, num_devices=8)

    d_in = {}
    for nm in ("uin_hi", "uin_lo"):
        d_in[nm] = nc.dram_tensor(nm, [C, QWIN], f16, kind="ExternalInput").ap()
    for nm in ("uref_hi", "uref_lo"):
        d_in[nm] = nc.dram_tensor(nm, [C, PW], f16, kind="ExternalInput").ap()
    idx_d = nc.dram_tensor("idx", [128, NQT], u32, kind="ExternalOutput").ap()
    junk_d = nc.dram_tensor("junk", [1, 8], f32, kind="ExternalOutput").ap()

    TERMS = [("uin_hi", "uref_hi"), ("uin_lo", "uref_hi"), ("uin_hi", "uref_lo")]

    def cview(t, py0):
        # [128, 62, 62] view of a [128, 4096] tile: cols 64*(py0+py)+px
        return t[:, 64 * py0: 64 * py0 + 3968].rearrange(
            "p (a b) -> p a b", b=64)[:, :, 0:62]

    with tile.TileContext(nc) as tc:
        with tc.tile_pool(name="const", bufs=1) as cpool, \
             tc.tile_pool(name="work2", bufs=2) as wpool, \
             tc.tile_pool(name="small", bufs=2) as spool, \
             tc.tile_pool(name="stage", bufs=1) as stpool, \
             tc.tile_pool(name="ps", bufs=1, space="PSUM") as ps:

            # Input DMAs; order matters for the wait-absorber below.
            # uref loads split by column half; all LEFT halves load first so
            # PE pieces 0-1 (all three terms) unblock as early as possible.
            ins = {}
            for nm, w in (("uin_hi", QWIN), ("uref_hi", PW),
                          ("uin_lo", QWIN), ("uref_lo", PW)):
                for ch in range(2):
                    t = cpool.tile([128, w], f16, name="dma_in", tag=f"{nm}{ch}")
                    ins[(nm, ch)] = t
                    if w == QWIN:
                        nc.sync.dma_start(t[:], d_in[nm][128 * ch:128 * (ch + 1), :])
                    else:
                        nc.sync.dma_start(t[:, 0:2048],
                                          d_in[nm][128 * ch:128 * (ch + 1), 0:2048])
            for nm in ("uref_hi", "uref_lo"):
                for ch in range(2):
                    nc.sync.dma_start(ins[(nm, ch)][:, 2048:PW],
                                      d_in[nm][128 * ch:128 * (ch + 1), 2048:PW])

            # Wait-absorber matmuls: walrus allows only one sync wait on the
            # LDW side of a matmul; absorb the uin-side DMA waits here so real
            # matmuls never carry two fresh DMA waits.
            junk_ps = ps.tile([128, 8], f32, name="junkps", tag="pspc3")
            regions = [ins[(nm, ch)]
                       for nm in ("uin_hi", "uin_lo")
                       for ch in range(2)]
            for i, r in enumerate(regions):
                nc.tensor.matmul(junk_ps[:1, :8], r[:, :1], r[:, :8],
                                 start=(i == 0), stop=(i == len(regions) - 1))
            junk_sb = stpool.tile([128, 8], f32, name="junksb", tag="junk")
            nc.vector.tensor_copy(junk_sb[:1, :8], junk_ps[:1, :8])

            idx_stage = stpool.tile([128, NQT], u32, name="idxs", tag="idxs")

            def assemble_main(t, prev, cur, mx, mi):
                # stage64[q] = C3[q+64, p+64], compact px, ACT rebase copies
                st64 = wpool.tile([128, NV], f32, name="st64", tag="st64")
                nc.scalar.copy(st64[0:64, :], cview(prev, 1)[64:128])
                nc.scalar.copy(st64[64:128, :], cview(cur, 1)[0:64])
                # t2 = C3[compact] + stage64 (in-place onto st64);
                # corr: t2 += C3_next[py+2 view]. Pool/DVE work concurrently.
                t2 = st64
                SP = 40 * 62   # 40 of 62 py-groups on Pool for t2
                SC = 40 * 62   # 40 of 62 py-groups on DVE for corr
                nc.gpsimd.tensor_add(t2[:, 0:SP], cview(prev, 0)[:, 0:40, :],
                                     st64[:, 0:SP])
                nc.vector.tensor_add(t2[:, SP:], cview(prev, 0)[:, 40:62, :],
                                     st64[:, SP:])
                nc.vector.tensor_add(t2[:, 0:SC], t2[:, 0:SC],
                                     cview(cur, 2)[:, 0:40, :])
                nc.gpsimd.tensor_add(t2[:, SC:], t2[:, SC:],
                                     cview(cur, 2)[:, 40:62, :])
                nc.vector.max(mx[:], t2[:])
                nc.vector.max_index(mi[:], mx[:], t2[:])
                nc.vector.tensor_copy(idx_stage[:, t:t + 1], mi[:, 0:1])

            c3_tiles = {}
            for ct in range(NCT):
                # --- PE: G tile ct in PSUM, four 2-bank pieces; ACT copies
                # each piece to SBUF right after its matmuls so the next
                # tile's PE piece is never blocked long (keeps PE p-state hot).
                gsb = wpool.tile([128, PW], f32, name="gsb", tag="gsb")
                for pc in range(4):
                    psp = ps.tile([128, 1024], f32, name=f"pspc{pc}",
                                  tag=f"pspc{pc}")
                    step = 0
                    for (anm, bnm) in TERMS:
                        for ch in range(2):
                            lhsT = ins[(anm, ch)][:, ct * 128: ct * 128 + 128]
                            for bk in range(2):
                                rhs = ins[(bnm, ch)][:, pc * 1024 + bk * 512:
                                                     pc * 1024 + bk * 512 + 512]
                                nc.tensor.matmul(
                                    psp[:, bk * 512:(bk + 1) * 512], lhsT, rhs,
                                    start=(step == 0), stop=(step == 5))
                            step += 1
                    for bk in range(2):
                        nc.scalar.copy(
                            gsb[:, pc * 1024 + bk * 512: pc * 1024 + (bk + 1) * 512],
                            psp[:, bk * 512:(bk + 1) * 512])

                # --- shift stages via DMA rebase, split L/R at the gsb
                # piece boundary so each half starts as soon as its source
                # pieces land: s1 = G^(+1,+1), s2 = G^(+2,+2).
                # Rows 126/127 junk (feed only qx in {62,63}, host-discarded).
                s1L = wpool.tile([128, 2047], f32, name="s1L", tag="s1L")
                s1R = wpool.tile([128, 2047], f32, name="s1R", tag="s1R")
                s2L = wpool.tile([128, 2046], f32, name="s2L", tag="s2L", bufs=1)
                s2R = wpool.tile([128, 2048], f32, name="s2R", tag="s2R", bufs=1)
                nc.sync.dma_start(s1L[0:127, :], gsb[1:128, 1:2048])
                nc.sync.dma_start(s1L[127:128, :], gsb[127:128, 1:2048])
                nc.scalar.dma_start(s2L[0:126, :], gsb[2:128, 2:2048])
                nc.scalar.dma_start(s2L[126:128, :], gsb[126:128, 2:2048])
                nc.sync.dma_start(s1R[0:127, :], gsb[1:128, 2048:W3 + 1])
                nc.sync.dma_start(s1R[127:128, :], gsb[127:128, 2048:W3 + 1])
                nc.scalar.dma_start(s2R[0:126, :], gsb[2:128, 2048:W3 + 2])
                nc.scalar.dma_start(s2R[126:128, :], gsb[126:128, 2048:W3 + 2])

                # --- a1 (GPSIMD) L/R: c3 = G + s1;  a2 (DVE, in-place) += s2
                c3 = wpool.tile([128, PW], f32, name="c3", tag="c3", bufs=3)
                nc.gpsimd.tensor_add(c3[:, 0:2047], gsb[:, 0:2047], s1L[:])
                nc.vector.tensor_add(c3[:, 0:2046], c3[:, 0:2046], s2L[:])
                nc.gpsimd.tensor_add(c3[:, 2047:W3], gsb[:, 2047:W3], s1R[:])
                nc.vector.tensor_add(c3[:, 2046:W3], c3[:, 2046:W3], s2R[:])
                c3_tiles[ct] = c3
                # 2-tile-lag assembly: every input of each queued op is >=1
                # iteration old, so no engine FIFO head-of-line blocks.
                if ct < 2:
                    continue
                t = ct - 2
                mx = spool.tile([128, 8], f32, name="mx", tag="mx")
                mi = spool.tile([128, 8], u32, name="mi", tag="mi")
                assemble_main(t, c3_tiles[t], c3_tiles[t + 1], mx, mi)
                del c3_tiles[t]

            mx = spool.tile([128, 8], f32, name="mxf", tag="mx")
            mi = spool.tile([128, 8], u32, name="mif", tag="mi")
            assemble_main(NCT - 2, c3_tiles[NCT - 2], c3_tiles[NCT - 1], mx, mi)

            nc.sync.dma_start(idx_d[:], idx_stage[:])
            nc.sync.dma_start(junk_d[:], junk_sb[:1, :8])

    nc.compile()
    return nc, run_bass_kernel_spmd


def _unit_pixels(f):
    # f: (C, H, W) float32; unit L2 norm per pixel across channels (fp32 math)
    n = np.sqrt(np.sum(f * f, axis=0, keepdims=True, dtype=np.float32))
    return (f / np.maximum(n, np.float32(1e-12))).astype(np.float32)


def _split_f16(a):
    hi = a.astype(np.float16)
    lo = (a - hi.astype(np.float32)).astype(np.float16)
    return hi, lo


def kernel(dense_features1, dense_features2, img_ref_hr):
    global _RUNNER
    if _RUNNER is None:
        _RUNNER = _build_runner()
    nc, run_spmd = _RUNNER

    f1 = np.asarray(dense_features1, dtype=np.float32)  # input features (b,C,H,W)
    f2 = np.asarray(dense_features2, dtype=np.float32)  # ref features
    B = f1.shape[0]
    assert B == 2 and f1.shape[1:] == (C, H, W)

    in_maps = []
    per_img = []
    for b in range(B):
        fin_u = _unit_pixels(f1[b]).reshape(C, HP)
        fref_u = _unit_pixels(f2[b]).reshape(C, HP)
        uin = np.zeros((C, QW_PAD), np.float32)
        uin[:, :HP] = fin_u
        uref = fref_u
        uin_hi, uin_lo = _split_f16(uin)
        uref_hi, uref_lo = _split_f16(uref)
        per_img.append((uin_hi, uin_lo, uref_hi, uref_lo))

    for core in range(8):
        b, qblk = divmod(core, 4)
        uin_hi, uin_lo, uref_hi, uref_lo = per_img[b]
        q0 = qblk * QBLK
        in_maps.append({
            "uin_hi": np.ascontiguousarray(uin_hi[:, q0:q0 + QWIN]),
            "uin_lo": np.ascontiguousarray(uin_lo[:, q0:q0 + QWIN]),
            "uref_hi": np.ascontiguousarray(uref_hi),
            "uref_lo": np.ascontiguousarray(uref_lo),
        })

    results = run_spmd(nc, in_maps, list(range(8))).results

    # Decode: idx_stage[part, tile] = argmax over compact (py, px) grid for
    # q_local = tile*128 + part, global q = core_q0 + q_local.
    out = np.zeros((B, 9, H, W, 2), np.float32)
    qx_grid = np.arange(62, dtype=np.float32)[None, :]
    qy_grid = np.arange(62, dtype=np.float32)[:, None]
    for b in range(B):
        idx_full = np.zeros(HP, np.int64)
        for qblk in range(4):
            r = results[b * 4 + qblk]["idx"]  # (128, NQT) uint32
            idx_full[qblk * QBLK:(qblk + 1) * QBLK] = r.T.reshape(-1)
        idx_grid = idx_full.reshape(H, W)[:62, :62]
        py = (idx_grid // 62).astype(np.float32)
        px = (idx_grid % 62).astype(np.float32)
        flow = np.zeros((H, W, 2), np.float32)
        flow[:62, :62, 0] = px - qx_grid
        flow[:62, :62, 1] = py - qy_grid
        for k, (i, j) in enumerate([(i, j) for i in range(3) for j in range(3)]):
            out[b, k, i:, j:, :] = flow[:H - i, :W - j, :]
    return out
